# revision 1
# baseline (speedup 1.0000x reference)
"""Trainium2 Bass kernel for nn_BGNLLLoss (bivariate-Gaussian NLL loss).

Math (per element t,p):
    mux,muy,lsx,lsy,pc = params[t,p,:];  x,y = targets[t,p,:]
    sx=e^lsx, sy=e^lsy, c=tanh(pc), nr=1-c^2
    a=(x-mux)/sx, b=(y-muy)/sy
    nll = min( (a^2+b^2-2abc)/(2nr) + lsx+lsy + 0.5 ln(nr) + ln(2pi),
               -ln(1e-20) )
    loss[p] = sum_t nll[t,p]

tanh-free reformulation (keeps ScalarE in ONE table set: exp+ln+square):
  t4  = e^{-2 pc}            =>  c = (1-t4)/(1+t4),  nr = 4 t4/(1+t4)^2
  gv  = a(1+t4) + b(t4-1)    =  (a - cb)(1+t4)
  (a^2+b^2-2abc)/(2nr)       =  gv^2 e^{2pc}/8 + b^2/2
  0.5 ln(nr)                 =  ln2 - pc - ln(1+t4)
  nll = min( (gv st)^2 + bh^2 + (lsx+lsy-pc) - lvc, K )
    with st = e^{pc}/(2 sqrt2), bh = b/sqrt2,
         lvc = ln(1+t4) - (ln2 + ln 2pi)  [folded into the Ln's scale/bias]

Engine split (per 256-row block; all 16 blocks pipelined by Tile):
  ScalarE: isx, isyh(=isy/sqrt2), t4, st, lvc          (5 ACTIVATEs)
  GpSimd : ny, s1=lsx+lsy, s1b=s1-pc                   (3 tensor ops)
  VectorE: bf16 2x chain a,bh,av,qn,gv,gvs,u,b2,W,V + 2 ts + 1 custom min
  TensorE: frame sum   acc[1,512] += ones^T @ nll      (2 matmuls)
Sharding: person dim split across 8 cores (512 each), no collectives.
"""

import json
import math
import os
import shutil
import tempfile
from contextlib import ExitStack

import numpy as np

import concourse.bass as bass
import concourse.bacc as bacc
import concourse.mybir as mybir
import concourse.tile as tile
from concourse import bass_utils
from concourse.dve_spec import Spec, Src0, Src1, C0, C1, lower, sq, minn, _has_src1
from concourse.dve_uop import DveOpSpec
import concourse.dve_ops as dve_ops

F32 = mybir.dt.float32
BF16 = mybir.dt.bfloat16
AF = mybir.ActivationFunctionType
ALU = mybir.AluOpType

T = 4096
P = 4096
N_CORES = 8
PC = P // N_CORES          # persons per core = 512
K = 2                      # 128-row subtiles per block
RB = 128 * K               # rows per block
NB = T // RB               # 16 blocks
TGT_W = PC * 2             # 1024
PRM_W = PC * 5             # 2560

LOG2PI = math.log(2.0 * math.pi)
LN2 = math.log(2.0)
CADD = LN2 + LOG2PI                    # additive const inside the min
CLAMP = -math.log(1e-20)               # 46.0517...
SQRT2 = math.sqrt(2.0)
B_ISYH = -0.5 * LN2                    # exp bias: isy/sqrt(2)
B_ST = -1.5 * LN2                      # exp bias: e^{pc}/(2 sqrt 2)
SC_LN = math.exp(-CADD)                # ln scale/bias: ln(1+t4) - CADD


# --------------------------------------------------------------------------
# Custom DVE op: out = min(in0 + in1 + s0, s1)
# --------------------------------------------------------------------------
def _register_dve_op(name: str, spec: Spec, subdim: bool = False):
    if name in dve_ops._SUB_OPCODE_FOR_NAME:
        return next(op for op in dve_ops.OPS if op.name == name)
    shas = {}
    for ver in ("v3", "v4"):
        uops = lower(spec, ver=ver)
        shas[ver] = DveOpSpec(
            name=name, opcode=0, uops=uops, rd1_en=_has_src1(spec)
        ).sha(ver)
    op = dve_ops.DveOp(name, spec, subdim=subdim, uops_sha=shas)
    dve_ops.OPS.append(op)
    dve_ops._SUB_OPCODE_FOR_NAME[name] = (
        dve_ops._CUSTOM_DVE_ROW_BASE + len(dve_ops.OPS) - 1
    )
    dve_ops.CUSTOM_DVE_SPECS[name] = spec
    return op


ADDMIN = _register_dve_op(
    "ADDMIN_BGNLL",
    Spec(
        body=minn(Src0 + Src1 + C0, C1),
        reference=lambda in0, in1, s0, s1, imm2: np.minimum(
            in0.astype(np.float32) + in1 + s0, s1
        ).astype(np.float32),
    ),
)

# out = sq(in0) + sq(in1)
SQ2 = _register_dve_op(
    "SQ2_BGNLL",
    Spec(
        body=sq(Src0) + sq(Src1),
        reference=lambda in0, in1, s0, s1, imm2: (
            np.square(in0.astype(np.float32)) + np.square(in1.astype(np.float32))
        ).astype(np.float32),
    ),
)

# Fast-log constants: for x = 2^e (1+f), int_bits(x)/2^23 = e + 127 + f and
# log2(x) = e + log2(1+f), so ln(x) ~= (int_bits(x) - SIGMA) * ln2/2^23 with
# the mantissa correction c = E[log2(1+f) - f] = 1.5 - 1/ln2 (zero-mean over
# uniform f) and the additive constant CADD both folded into SIGMA.
LNK = math.log(2.0) / (1 << 23)
_C_MEAN = 1.5 - 1.0 / math.log(2.0)            # 0.0573049...
SIGMA_F = (127.0 - _C_MEAN + CADD / math.log(2.0)) * (1 << 23)



# --------------------------------------------------------------------------
# ACT table-set fix: walrus assigns Exp -> exp_and_others and Ln ->
# natural_log_exp_and_others, reloading tables every block (~2.6us/block).
# Reorder act_info.json so the combined exp+ln set is found first for both.
# --------------------------------------------------------------------------
def _install_act_json():
    if os.environ.get("BGNLL_NO_ACT_JSON"):
        return
    if os.environ.get("BASS_ACT_ROOT_JSON_PATH"):
        return
    try:
        from neuronxcc.driver.Job import Job
        from neuronxcc.driver.jobs.support.FindActInfo import findActInfoFile
        src = findActInfoFile(Job.getPackageDir(), "gen3")
    except Exception:
        return
    if not src:
        return
    src_dir = os.path.dirname(src)
    dst_dir = os.path.join(tempfile.gettempdir(), "bgnll_act_root")
    os.makedirs(dst_dir, exist_ok=True)
    with open(src) as f:
        info = json.load(f)
    sets = info.get("act_func_sets", [])
    pref = [s for s in sets if s.get("name") == "natural_log_exp_and_others"]
    rest = [s for s in sets if s.get("name") != "natural_log_exp_and_others"]
    if not pref:
        return
    info["act_func_sets"] = pref + rest
    for name in os.listdir(src_dir):
        s = os.path.join(src_dir, name)
        d = os.path.join(dst_dir, name)
        if os.path.isfile(s) and not os.path.exists(d) and name != "act_info.json":
            try:
                os.symlink(s, d)
            except OSError:
                shutil.copy(s, d)
    with open(os.path.join(dst_dir, "act_info.json"), "w") as f:
        json.dump(info, f)
    os.environ["BASS_ACT_ROOT_JSON_PATH"] = os.path.join(dst_dir, "act_info.json")


# --------------------------------------------------------------------------
# Kernel body (per core; SPMD -- same program on all 8 cores)
# --------------------------------------------------------------------------
def _emit(ctx: ExitStack, tc: tile.TileContext, tgt: bass.AP, prm: bass.AP,
          loss: bass.AP):
    nc = tc.nc

    iot = ctx.enter_context(tc.tile_pool(name="iot", bufs=3))
    iop = ctx.enter_context(tc.tile_pool(name="iop", bufs=4))
    tp = ctx.enter_context(tc.tile_pool(name="tp", bufs=3))
    tp2 = ctx.enter_context(tc.tile_pool(name="tp2", bufs=2))
    single = ctx.enter_context(tc.tile_pool(name="single", bufs=1))
    psum_pool = ctx.enter_context(
        tc.tile_pool(name="psum", bufs=1, space="PSUM")
    )

    ones = single.tile([128, 1], F32)
    nc.vector.memset(ones[:], 1.0)
    acc = psum_pool.tile([1, PC], F32)

    shb = [128, K, PC]
    ctxs: dict[int, dict] = {}

    def stage_load(blk):
        r0 = blk * RB
        tgv = tgt[r0:r0 + RB, :].rearrange("(k p) w -> p k w", k=K, p=128)
        prv = prm[r0:r0 + RB, :].rearrange("(k p) w -> p k w", k=K, p=128)
        tg = iot.tile([128, K, TGT_W], F32, tag="tg")
        nc.sync.dma_start(tg[:], tgv)
        pr = iop.tile([128, K, PRM_W], F32, tag="pr")
        nc.sync.dma_start(pr[:], prv)
        ctxs[blk] = {"tg": tg, "pr": pr}

    def stage_front(blk):
        c = ctxs[blk]
        tg4 = c["tg"][:].rearrange("p k (n c) -> p k n c", c=2)
        pr4 = c["pr"][:].rearrange("p k (n c) -> p k n c", c=5)
        c["t0v"], c["t1v"] = tg4[:, :, :, 0], tg4[:, :, :, 1]
        c["p0v"], c["p1v"] = pr4[:, :, :, 0], pr4[:, :, :, 1]
        p2v, p3v, p4v = pr4[:, :, :, 2], pr4[:, :, :, 3], pr4[:, :, :, 4]
        c["p2v"], c["p3v"], c["p4v"] = p2v, p3v, p4v

        t4 = tp.tile(shb, F32, tag="t4")
        t4p1f = tp.tile(shb, F32, tag="t4p1f")
        t4m1s = tp.tile(shb, BF16, tag="t4m1s")
        isx = tp.tile(shb, BF16, tag="isx")
        isyh = tp.tile(shb, BF16, tag="isyh")
        st = tp.tile(shb, BF16, tag="st")
        lvc = tp.tile(shb, BF16, tag="lvc")
        B = tp.tile(shb, BF16, tag="B")      # nyt -> bh
        S = tp.tile(shb, F32, tag="S")       # s1 -> s1b
        c.update(t4=t4, t4p1f=t4p1f, t4m1s=t4m1s, isx=isx, isyh=isyh,
                 st=st, lvc=lvc, B=B, S=S)

        # --- ScalarE: Exp-only (single table set) + affines ---
        nc.scalar.activation(t4[:], p4v, AF.Exp, scale=-2.0)
        nc.scalar.activation(t4p1f[:], t4[:], AF.Identity, scale=1.0,
                             bias=1.0)
        nc.scalar.activation(t4m1s[:], t4[:], AF.Identity, scale=SQRT2,
                             bias=-SQRT2)
        nc.scalar.activation(isx[:], p2v, AF.Exp, scale=-1.0)
        nc.scalar.activation(isyh[:], p3v, AF.Exp, scale=-1.0, bias=B_ISYH)
        nc.scalar.activation(st[:], p4v, AF.Exp, scale=1.0, bias=B_ST)
        # lvc = ln(1+t4) - CADD via the exponent-bits log approximation:
        # int32 bits of t4p1f, converted + affine-mapped in one ACTIVATE.
        nc.scalar.activation(lvc[:], t4p1f[:].bitcast(mybir.dt.int32),
                             AF.Identity, scale=LNK, bias=-SIGMA_F * LNK)

        # --- GpSimd: the fp32 strided side-chain ---
        nc.gpsimd.tensor_sub(B[:], c["t1v"], c["p1v"])        # nyt
        nc.gpsimd.tensor_add(S[:], p2v, p3v)                  # s1
        nc.gpsimd.tensor_sub(S[:], S[:], p4v)                 # s1b

    def stage_dve(blk):
        c = ctxs[blk]
        A = tp.tile(shb, BF16, tag="A")      # nxt -> a
        G = tp2.tile(shb, BF16, tag="G")     # av -> gv -> gvs
        qn = tp2.tile(shb, BF16, tag="qn")
        W = tp2.tile(shb, BF16, tag="W")
        VN = tp2.tile(shb, F32, tag="VN")    # V -> nll
        B, S = c["B"], c["S"]

        nc.vector.tensor_sub(A[:], c["t0v"], c["p0v"])        # nxt
        nc.vector.tensor_mul(A[:], A[:], c["isx"][:])         # a
        nc.vector.tensor_mul(B[:], B[:], c["isyh"][:])        # bh
        nc.vector.tensor_mul(G[:], A[:], c["t4p1f"][:])       # av
        nc.vector.tensor_mul(qn[:], B[:], c["t4m1s"][:])
        nc.vector.tensor_add(G[:], G[:], qn[:])               # gv
        nc.vector.tensor_mul(G[:], G[:], c["st"][:])          # gvs
        Wf = W[:].rearrange("p k n -> p (k n)")
        nc.vector._custom_dve(SQ2, out=Wf,
                              in0=G[:].rearrange("p k n -> p (k n)"),
                              in1=B[:].rearrange("p k n -> p (k n)"))
        nc.vector.tensor_sub(VN[:], W[:], c["lvc"][:])        # V
        Vf = VN[:].rearrange("p k n -> p (k n)")
        Sf = S[:].rearrange("p k n -> p (k n)")
        nc.vector._custom_dve(ADDMIN, out=Vf, in0=Vf, in1=Sf, s0=0.0,
                              s1=CLAMP)

        # --- TensorE: frame sum ---
        for k in range(K):
            nc.tensor.matmul(
                acc[:, :], ones[:, :], VN[:, k, :],
                start=(blk == 0 and k == 0),
                stop=(blk == NB - 1 and k == K - 1),
            )
        del ctxs[blk]

    # Skewed emission (software pipelining): DMA for blk+2, producers for
    # blk+1, consumers for blk — gives the static scheduler cross-block
    # interleaving priority.
    for i in range(NB + 2):
        if i < NB:
            stage_load(i)
        if 1 <= i and i - 1 < NB:
            stage_front(i - 1)
        if 2 <= i and i - 2 < NB:
            stage_dve(i - 2)

    out_sb = single.tile([1, PC], F32)
    nc.vector.tensor_copy(out_sb[:], acc[:, :])
    nc.sync.dma_start(loss, out_sb[:])


_CACHED_NC = None


def _build_program() -> bass.Bass:
    global _CACHED_NC
    if _CACHED_NC is not None:
        return _CACHED_NC
    nc = bacc.Bacc("TRN2", target_bir_lowering=False, debug=False,
                   enable_asserts=False)
    for v in (B_ISYH, B_ST, -SQRT2, -SIGMA_F * LNK):
        t = nc.alloc_sbuf_tensor(f"const-f32-{v}", [128, 1], F32)
        nc.gpsimd.memset(t.ap(), v)
        nc.const_aps.aps[(F32, v)] = t.ap()
    nc.all_engine_barrier()
    tgt = nc.dram_tensor("tgt", [T, TGT_W], F32, kind="ExternalInput").ap()
    prm = nc.dram_tensor("prm", [T, PRM_W], F32, kind="ExternalInput").ap()
    loss = nc.dram_tensor("loss", [1, PC], F32, kind="ExternalOutput").ap()
    with tile.TileContext(nc) as tc:
        with ExitStack() as ctx:
            _emit(ctx, tc, tgt, prm, loss)
    nc.compile()
    _CACHED_NC = nc
    return nc


def make_in_maps(targets: np.ndarray, params: np.ndarray):
    targets = np.asarray(targets, dtype=np.float32)
    params = np.asarray(params, dtype=np.float32)
    in_maps = []
    for i in range(N_CORES):
        sl = slice(i * PC, (i + 1) * PC)
        in_maps.append({
            "tgt": np.ascontiguousarray(targets[:, sl, :]).reshape(T, TGT_W),
            "prm": np.ascontiguousarray(params[:, sl, :]).reshape(T, PRM_W),
        })
    return in_maps


def run_spmd(targets: np.ndarray, params: np.ndarray, trace: bool = False):
    nc = _build_program()
    in_maps = make_in_maps(targets, params)
    res = bass_utils.run_bass_kernel_spmd(
        nc, in_maps, core_ids=list(range(N_CORES)), trace=trace,
    )
    loss = np.concatenate(
        [res.results[i]["loss"].reshape(PC) for i in range(N_CORES)]
    ).astype(np.float32)
    return loss, res


def kernel(targets: np.ndarray, params: np.ndarray,
           peopleIDs: np.ndarray | None = None) -> np.ndarray:
    loss, _ = run_spmd(targets, params, trace=False)
    return loss



# revision 3
# speedup vs baseline: 1.8630x; 1.8630x over previous
"""Trainium2 Bass kernel for nn_BGNLLLoss (bivariate-Gaussian NLL loss).

Math (per element t,p):
    mux,muy,lsx,lsy,pc = params[t,p,:];  x,y = targets[t,p,:]
    sx=e^lsx, sy=e^lsy, c=tanh(pc), nr=1-c^2
    a=(x-mux)/sx, b=(y-muy)/sy
    nll = min( (a^2+b^2-2abc)/(2nr) + lsx+lsy + 0.5 ln(nr) + ln(2pi),
               -ln(1e-20) )
    loss[p] = sum_t nll[t,p]

tanh-free identity (single Exp table set; ln via exponent-bits trick):
  t4  = e^{-2 pc};  (a^2+b^2-2abc)/(2nr) = gvs^2 + bh^2
    with gv = a*(1+t4) + bh*sqrt2*(t4-1)   [bh = b/sqrt2]
         gvs = gv * e^{pc}/(2 sqrt2)
  0.5 ln(nr) = ln2 - pc - ln(1+t4)
  nll = min( gvs^2 + bh^2 + (lsx+lsy-pc) - lvc, CLAMP )
    with lvc = ln(1+t4) - (ln2 + ln 2pi), computed from the bf16 bit
    pattern of (1+t4): ln(u) ~ (int16_bits(u)/2^7 - sigma)*ln2.

Layout/engine plan (per core; SPMD on 8 cores, persons sharded):
  Host de-interleaves the 7 channels [x,y,mux,muy,lsx,lsy,pc] into a
  person-major bf16 array [8 blocks, 128 persons, 7, 2048 frames]
  (halves HBM traffic; every SBUF operand is contiguous).
  Persons sit on partitions, so the frame-sum is a free accum_out on
  the final fused DVE op -- no TensorE matmuls, no PSUM.
    ScalarE: t4, t4p1, lvc(bits), isx, isyh, st    (6 ACTIVATEs)
    GpSimd : s1=lsx+lsy, s2=s1-pc                  (2 tensor ops)
    VectorE: t4m1s (TS) | nx,ny,a,bh,av,qn,gv,gvs,S2c (9 TT)
             WS2 = bh^2 + S2c                    (custom)
             minn(gvs^2+WS2, CLAMP)              (custom, accum->loss)
"""

import math
from contextlib import ExitStack

import numpy as np
import ml_dtypes

import concourse.bass as bass
import concourse.bacc as bacc
import concourse.mybir as mybir
import concourse.tile as tile
from concourse import bass_utils
from concourse.dve_spec import Spec, Src0, Src1, C0, C1, lower, sq, minn, _has_src1
from concourse.dve_spec import AluOp
from concourse.dve_uop import DveOpSpec
import concourse.dve_ops as dve_ops

F32 = mybir.dt.float32
BF16 = mybir.dt.bfloat16
I16 = mybir.dt.int16
AF = mybir.ActivationFunctionType
ALU = mybir.AluOpType
BF16NP = ml_dtypes.bfloat16

T = 4096
P = 4096
N_CORES = 8
PC = P // N_CORES          # persons per core = 512
NG = PC // 128             # person groups of 128 = 4
FT = 2048                  # frames per block
NT = T // FT               # 2 frame chunks
NBLK = NG * NT             # 8 blocks
CW = 7 * FT                # per-partition row: 7 channels x FT = 14336

LOG2PI = math.log(2.0 * math.pi)
LN2 = math.log(2.0)
CADD = LN2 + LOG2PI                    # additive const inside the min
CLAMP = -math.log(1e-20)               # 46.0517...
SQRT2 = math.sqrt(2.0)
B_ISYH = -0.5 * LN2                    # exp bias: isy/sqrt(2)
B_ST = -1.5 * LN2                      # exp bias: e^{pc}/(2 sqrt 2)

# Fast-log constants (bf16 variant): for u = 2^e (1+f) >= 1, the bf16 bit
# pattern as int16 is bits = (e+127)*2^7 + f*2^7, so ln(u) ~=
# (bits/2^7 - (127 - c))*ln2 with the mantissa correction c = E[log2(1+f)-f]
# = 1.5 - 1/ln2. CADD is folded into sigma so lvc = ln(1+t4) - CADD.
LNK16 = LN2 / (1 << 7)
_C_MEAN = 1.5 - 1.0 / LN2
SIGMA16 = (127.0 - _C_MEAN + CADD / LN2) * (1 << 7)
B_LVC = -SIGMA16 * LNK16


# --------------------------------------------------------------------------
# Custom DVE ops
# --------------------------------------------------------------------------
def _register_dve_op(name: str, spec: Spec, subdim: bool = False):
    if name in dve_ops._SUB_OPCODE_FOR_NAME:
        return next(op for op in dve_ops.OPS if op.name == name)
    shas = {}
    for ver in ("v3", "v4"):
        uops = lower(spec, ver=ver)
        shas[ver] = DveOpSpec(
            name=name, opcode=0, uops=uops, rd1_en=_has_src1(spec)
        ).sha(ver)
    op = dve_ops.DveOp(name, spec, subdim=subdim, uops_sha=shas)
    dve_ops.OPS.append(op)
    dve_ops._SUB_OPCODE_FOR_NAME[name] = (
        dve_ops._CUSTOM_DVE_ROW_BASE + len(dve_ops.OPS) - 1
    )
    dve_ops.CUSTOM_DVE_SPECS[name] = spec
    return op


# WS2 = bh^2 + S2c
SQADD = _register_dve_op("SQADD_BGNLL", Spec(body=sq(Src0) + Src1))

# nll = min(gvs^2 + WS2 + C0, C1); accum_out = sum(nll) over free dim
NLLSUM = _register_dve_op(
    "NLLSUM_BGNLL",
    Spec(body=minn(sq(Src0) + Src1 + C0, C1), accum=AluOp.ADD),
)


# --------------------------------------------------------------------------
# Kernel body (per core; SPMD -- same program on all 8 cores)
# --------------------------------------------------------------------------
def _emit(ctx: ExitStack, tc: tile.TileContext, inp: bass.AP, loss: bass.AP):
    nc = tc.nc

    iop = ctx.enter_context(tc.tile_pool(name="iop", bufs=3))
    sp = ctx.enter_context(tc.tile_pool(name="sp", bufs=2))
    gp = ctx.enter_context(tc.tile_pool(name="gp", bufs=2))
    tp = ctx.enter_context(tc.tile_pool(name="tp", bufs=1))
    single = ctx.enter_context(tc.tile_pool(name="single", bufs=1))

    part = single.tile([128, NBLK], F32)
    out_sb = single.tile([128, NG], F32)

    sh = [128, FT]
    ctxs: dict[int, dict] = {}

    def stage_load(blk):
        io = iop.tile([128, CW], BF16, tag="io")
        nc.sync.dma_start(io[:], inp[blk * 128:(blk + 1) * 128, :])
        ctxs[blk] = {"io": io}

    def stage_front(blk):
        c = ctxs[blk]
        io = c["io"]
        xv, yv, muxv, muyv, lsxv, lsyv, pcv = (
            io[:, i * FT:(i + 1) * FT] for i in range(7)
        )
        c.update(xv=xv, yv=yv, muxv=muxv, muyv=muyv, pcv=pcv)

        t4 = sp.tile(sh, BF16, tag="t4")
        t4p1 = sp.tile(sh, BF16, tag="t4p1")
        lvc = sp.tile(sh, BF16, tag="lvc")
        isx = sp.tile(sh, BF16, tag="isx")
        isyh = sp.tile(sh, BF16, tag="isyh")
        st = sp.tile(sh, BF16, tag="st")
        nc.scalar.activation(t4[:], pcv, AF.Exp, scale=-2.0)
        nc.scalar.activation(t4p1[:], t4[:], AF.Identity, scale=1.0, bias=1.0)
        nc.scalar.activation(lvc[:], t4p1[:].bitcast(I16), AF.Identity,
                             scale=LNK16, bias=B_LVC)
        nc.scalar.activation(isx[:], lsxv, AF.Exp, scale=-1.0)
        nc.scalar.activation(isyh[:], lsyv, AF.Exp, scale=-1.0, bias=B_ISYH)
        nc.scalar.activation(st[:], pcv, AF.Exp, scale=1.0, bias=B_ST)

        s1 = gp.tile(sh, BF16, tag="s1")
        s2 = gp.tile(sh, BF16, tag="s2")
        nc.gpsimd.tensor_add(s1[:], lsxv, lsyv)
        nc.gpsimd.tensor_sub(s2[:], s1[:], pcv)
        c.update(t4=t4, t4p1=t4p1, lvc=lvc, isx=isx, isyh=isyh, st=st, s2=s2)

    def stage_main(blk):
        c = ctxs[blk]
        t4m1s = tp.tile(sh, BF16, tag="t4m1s")
        nx = tp.tile(sh, BF16, tag="nx")
        ny = tp.tile(sh, BF16, tag="ny")
        a = tp.tile(sh, BF16, tag="a")
        bh = tp.tile(sh, BF16, tag="bh")
        av = tp.tile(sh, BF16, tag="av")
        qn = tp.tile(sh, BF16, tag="qn")
        gg = tp.tile(sh, BF16, tag="gg")
        s2c = tp.tile(sh, BF16, tag="s2c")
        ws2 = tp.tile(sh, BF16, tag="ws2")
        dead = tp.tile(sh, BF16, tag="dead")

        nc.vector.tensor_scalar(t4m1s[:], c["t4"][:], SQRT2, SQRT2,
                                op0=ALU.mult, op1=ALU.subtract)
        nc.vector.tensor_sub(nx[:], c["xv"], c["muxv"])
        nc.vector.tensor_sub(ny[:], c["yv"], c["muyv"])
        nc.vector.tensor_mul(a[:], nx[:], c["isx"][:])
        nc.vector.tensor_mul(bh[:], ny[:], c["isyh"][:])
        nc.vector.tensor_mul(av[:], a[:], c["t4p1"][:])
        nc.vector.tensor_mul(qn[:], bh[:], t4m1s[:])
        nc.vector.tensor_add(gg[:], av[:], qn[:])          # gv
        nc.vector.tensor_mul(av[:], gg[:], c["st"][:])     # gvs (reuse av)
        nc.vector.tensor_sub(s2c[:], c["s2"][:], c["lvc"][:])
        nc.vector._custom_dve(SQADD, out=ws2[:], in0=bh[:], in1=s2c[:])
        nc.vector._custom_dve(
            NLLSUM, out=dead[:], in0=av[:], in1=ws2[:],
            s0=0.0, s1=CLAMP, accum_out=part[:, blk:blk + 1],
        )
        del ctxs[blk]

    # Skewed emission (software pipelining): DMA for blk+2, front-end
    # producers for blk+1, vector consumers for blk.
    for i in range(NBLK + 2):
        if i < NBLK:
            stage_load(i)
        if 1 <= i and i - 1 < NBLK:
            stage_front(i - 1)
        if 2 <= i and i - 2 < NBLK:
            stage_main(i - 2)

    # loss[p, g] = part[:, 2g] + part[:, 2g+1]
    for g in range(NG):
        nc.vector.tensor_add(
            out_sb[:, g:g + 1], part[:, 2 * g:2 * g + 1],
            part[:, 2 * g + 1:2 * g + 2],
        )
    nc.sync.dma_start(loss, out_sb[:])


_CACHED_NC = None


def _build_program() -> bass.Bass:
    global _CACHED_NC
    if _CACHED_NC is not None:
        return _CACHED_NC
    nc = bacc.Bacc("TRN2", target_bir_lowering=False, debug=False,
                   enable_asserts=False)
    for v in (B_ISYH, B_ST, 1.0, B_LVC):
        t = nc.alloc_sbuf_tensor(f"const-f32-{v}", [128, 1], F32)
        nc.gpsimd.memset(t.ap(), v)
        nc.const_aps.aps[(F32, v)] = t.ap()
    nc.all_engine_barrier()
    inp = nc.dram_tensor("inp", [NBLK * 128, CW], BF16,
                         kind="ExternalInput").ap()
    loss = nc.dram_tensor("loss", [128, NG], F32, kind="ExternalOutput").ap()
    with tile.TileContext(nc) as tc:
        with ExitStack() as ctx:
            _emit(ctx, tc, inp, loss)
    nc.compile()
    _CACHED_NC = nc
    return nc


def make_in_maps(targets: np.ndarray, params: np.ndarray):
    targets = np.asarray(targets, dtype=np.float32)
    params = np.asarray(params, dtype=np.float32)
    in_maps = []
    for ci in range(N_CORES):
        sl = slice(ci * PC, (ci + 1) * PC)
        chans = (
            targets[:, sl, 0], targets[:, sl, 1],
            params[:, sl, 0], params[:, sl, 1],
            params[:, sl, 2], params[:, sl, 3], params[:, sl, 4],
        )
        arr = np.empty((NG, NT, 128, 7, FT), dtype=BF16NP)
        for k, ch in enumerate(chans):
            v = ch.astype(BF16NP)                    # [T, PC]
            vv = v.reshape(NT, FT, NG, 128)          # [tc, t', g, p]
            arr[:, :, :, k, :] = vv.transpose(2, 0, 3, 1)
        in_maps.append({"inp": arr.reshape(NBLK * 128, CW)})
    return in_maps


def run_spmd(targets: np.ndarray, params: np.ndarray, trace: bool = False):
    nc = _build_program()
    in_maps = make_in_maps(targets, params)
    res = bass_utils.run_bass_kernel_spmd(
        nc, in_maps, core_ids=list(range(N_CORES)), trace=trace,
    )
    # results[i]["loss"][p, g] is person g*128+p of core i's slice
    loss = np.concatenate(
        [np.asarray(res.results[i]["loss"]).astype(np.float32).T.ravel()
         for i in range(N_CORES)]
    )
    return loss, res


def kernel(targets: np.ndarray, params: np.ndarray,
           peopleIDs: np.ndarray | None = None) -> np.ndarray:
    loss, _ = run_spmd(targets, params, trace=False)
    return loss


# revision 6
# speedup vs baseline: 3.0645x; 1.6449x over previous
"""Trainium2 Bass kernel for nn_BGNLLLoss (bivariate-Gaussian NLL loss).

Math (per element t,p):
    mux,muy,lsx,lsy,pc = params[t,p,:];  x,y = targets[t,p,:]
    sx=e^lsx, sy=e^lsy, c=tanh(pc), nr=1-c^2
    a=(x-mux)/sx, b=(y-muy)/sy
    nll = min( (a^2+b^2-2abc)/(2nr) + lsx+lsy + 0.5 ln(nr) + ln(2pi),
               -ln(1e-20) )
    loss[p] = sum_t nll[t,p]

tanh-free identity (single Exp table set; ln via exponent-bits trick):
  t4  = e^{-2 pc};  (a^2+b^2-2abc)/(2nr) = gvs^2 + bh^2
    with gv = a*(1+t4) + bh*sqrt2*(t4-1)   [bh = b/sqrt2]
         gvs = gv * e^{pc}/(2 sqrt2)
  0.5 ln(nr) = ln2 - pc - ln(1+t4)
  nll = min( gvs^2 + W2, CLAMP ),  W2 = lsx+lsy-pc-lvc+bh^2
    with lvc = ln(1+t4) - (ln2 + ln 2pi), computed from the bf16 bit
    pattern of (1+t4): ln(u) ~ (int16_bits(u)/2^7 - sigma)*ln2.

Layout/engine plan (per core; SPMD on 8 cores, persons sharded):
  Host de-interleaves the 7 channels [x,y,mux,muy,lsx,lsy,pc] into a
  person-major bf16 array [8 blocks, 128 persons, 7, 2048 frames]
  (halves HBM traffic; every SBUF operand is contiguous).
  Persons sit on partitions, so the frame-sum is a free accum_out on
  the final fused DVE op.
    ScalarE: t4, lvc(bits), isx, isyh, st, bsq=bh^2   (6 ACTIVATEs)
    TensorE: W2 = I@lsx + I@lsy - I@pc - I@lvc + I@bsq
             (5 identity-weight matmuls accumulating in PSUM)
    VectorE: t4p1,t4m1s (TS 4x) | nx,ny,a,bh,av,qn,gv,gvs (8 TT 2x)
             minn(gvs^2 + W2, CLAMP) (custom, accum -> per-person sum)
    GpSimd : nothing (Pool shares the DVE SBUF port; keep it quiet)
"""

import math
from contextlib import ExitStack

import numpy as np
import ml_dtypes

import concourse.bass as bass
import concourse.bacc as bacc
import concourse.mybir as mybir
import concourse.tile as tile
from concourse import bass_utils
from concourse.dve_spec import Spec, Src0, Src1, C0, C1, lower, sq, minn, _has_src1
from concourse.dve_spec import AluOp
from concourse.dve_uop import DveOpSpec
import concourse.dve_ops as dve_ops

F32 = mybir.dt.float32
BF16 = mybir.dt.bfloat16
I16 = mybir.dt.int16
AF = mybir.ActivationFunctionType
ALU = mybir.AluOpType
BF16NP = ml_dtypes.bfloat16

T = 4096
P = 4096
N_CORES = 8
PC = P // N_CORES          # persons per core = 512
NG = PC // 128             # person groups of 128 = 4
FT = 2048                  # frames per block
NT = T // FT               # 2 frame chunks
NBLK = NG * NT             # 8 blocks
CW = 7 * FT                # per-partition row: 7 channels x FT = 14336

LOG2PI = math.log(2.0 * math.pi)
LN2 = math.log(2.0)
CADD = LN2 + LOG2PI                    # additive const inside the min
CLAMP = -math.log(1e-20)               # 46.0517...
SQRT2 = math.sqrt(2.0)
B_ISYH = -0.5 * LN2                    # exp bias: isy/sqrt(2)
B_ST = -1.5 * LN2                      # exp bias: e^{pc}/(2 sqrt 2)

# Fast-log constants (bf16 variant): for u = 2^e (1+f) >= 1, the bf16 bit
# pattern as int16 is bits = (e+127)*2^7 + f*2^7, so ln(u) ~=
# (bits/2^7 - (127 - c))*ln2 with the mantissa correction c = E[log2(1+f)-f]
# = 1.5 - 1/ln2. CADD is folded into sigma so lvc = ln(1+t4) - CADD.
LNK16 = LN2 / (1 << 7)
_C_MEAN = 1.5 - 1.0 / LN2
SIGMA16 = (127.0 - _C_MEAN + CADD / LN2) * (1 << 7)
B_LVC = -SIGMA16 * LNK16


# --------------------------------------------------------------------------
# Custom DVE op: nll = min(gvs^2 + W2 + C0, C1); accum_out = sum over frames
# --------------------------------------------------------------------------
def _register_dve_op(name: str, spec: Spec, subdim: bool = False):
    if name in dve_ops._SUB_OPCODE_FOR_NAME:
        return next(op for op in dve_ops.OPS if op.name == name)
    shas = {}
    for ver in ("v3", "v4"):
        uops = lower(spec, ver=ver)
        shas[ver] = DveOpSpec(
            name=name, opcode=0, uops=uops, rd1_en=_has_src1(spec)
        ).sha(ver)
    op = dve_ops.DveOp(name, spec, subdim=subdim, uops_sha=shas)
    dve_ops.OPS.append(op)
    dve_ops._SUB_OPCODE_FOR_NAME[name] = (
        dve_ops._CUSTOM_DVE_ROW_BASE + len(dve_ops.OPS) - 1
    )
    dve_ops.CUSTOM_DVE_SPECS[name] = spec
    return op


NLLSUM = _register_dve_op(
    "NLLSUM_BGNLL",
    Spec(body=minn(sq(Src0) + Src1 + C0, C1), accum=AluOp.ADD),
)


# --------------------------------------------------------------------------
# Kernel body (per core; SPMD -- same program on all 8 cores)
# --------------------------------------------------------------------------
def _emit(ctx: ExitStack, tc: tile.TileContext, inp: bass.AP, ident: bass.AP,
          loss: bass.AP):
    nc = tc.nc

    iop = ctx.enter_context(tc.tile_pool(name="iop", bufs=3))
    sp = ctx.enter_context(tc.tile_pool(name="sp", bufs=2))
    tp = ctx.enter_context(tc.tile_pool(name="tp", bufs=1))
    single = ctx.enter_context(tc.tile_pool(name="single", bufs=1))
    pp = ctx.enter_context(tc.tile_pool(name="pp", bufs=2, space="PSUM"))

    part = single.tile([128, NBLK], F32)
    out_sb = single.tile([128, NG], F32)
    id_sb = single.tile([128, 256], BF16)
    nc.sync.dma_start(id_sb[:], ident)
    idP = id_sb[:, 0:128]      # +identity weights
    idN = id_sb[:, 128:256]    # -identity weights

    sh = [128, FT]
    ctxs: dict[int, dict] = {}

    def stage_load(blk):
        io = iop.tile([128, CW], BF16, tag="io")
        nc.sync.dma_start(io[:], inp[blk * 128:(blk + 1) * 128, :])
        ctxs[blk] = {"io": io}

    def stage_front(blk):
        c = ctxs[blk]
        io = c["io"]
        xv, yv, muxv, muyv, lsxv, lsyv, pcv = (
            io[:, i * FT:(i + 1) * FT] for i in range(7)
        )
        c.update(xv=xv, yv=yv, muxv=muxv, muyv=muyv)

        t4 = sp.tile(sh, BF16, tag="t4")
        t4p1 = sp.tile(sh, BF16, tag="t4p1")
        lvc = sp.tile(sh, BF16, tag="lvc")
        isx = sp.tile(sh, BF16, tag="isx")
        isyh = sp.tile(sh, BF16, tag="isyh")
        st = sp.tile(sh, BF16, tag="st")
        nc.scalar.activation(t4[:], pcv, AF.Exp, scale=-2.0)
        nc.vector.tensor_scalar_add(t4p1[:], t4[:], 1.0)
        nc.scalar.activation(lvc[:], t4p1[:].bitcast(I16), AF.Identity,
                             scale=LNK16, bias=B_LVC)
        nc.scalar.activation(isx[:], lsxv, AF.Exp, scale=-1.0)
        nc.scalar.activation(isyh[:], lsyv, AF.Exp, scale=-1.0, bias=B_ISYH)
        nc.scalar.activation(st[:], pcv, AF.Exp, scale=1.0, bias=B_ST)

        # W2 partial sums on the (otherwise idle) PE array:
        # w2 = lsx + lsy - pc - lvc  (+ bsq later, in stage_main)
        # One matmul per 512-col PSUM bank; term-outer order keeps the
        # weight loads at 3 per block (P,P,N,N then P in stage_main).
        w2 = pp.tile(sh, F32, tag="w2")
        for w, src, st_, sp_ in ((idP, lsxv, True, False),
                                 (idP, lsyv, False, False),
                                 (idN, pcv, False, False),
                                 (idN, lvc[:], False, False)):
            for k in range(0, FT, 512):
                nc.tensor.matmul(w2[:, k:k + 512], w, src[:, k:k + 512],
                                 start=st_, stop=sp_)
        c.update(t4=t4, t4p1=t4p1, isx=isx, isyh=isyh, st=st, w2=w2)

    def stage_main(blk):
        c = ctxs[blk]
        t4m1s = tp.tile(sh, BF16, tag="t4m1s")
        nx = tp.tile(sh, BF16, tag="nx")
        ny = tp.tile(sh, BF16, tag="ny")
        a = tp.tile(sh, BF16, tag="a")
        bh = tp.tile(sh, BF16, tag="bh")
        bsq = tp.tile(sh, BF16, tag="bsq")
        av = tp.tile(sh, BF16, tag="av")
        qn = tp.tile(sh, BF16, tag="qn")
        gg = tp.tile(sh, BF16, tag="gg")
        dead = tp.tile(sh, BF16, tag="dead")

        nc.vector.tensor_scalar(t4m1s[:], c["t4"][:], SQRT2, SQRT2,
                                op0=ALU.mult, op1=ALU.subtract)
        nc.vector.tensor_sub(nx[:], c["xv"], c["muxv"])
        nc.vector.tensor_sub(ny[:], c["yv"], c["muyv"])
        nc.vector.tensor_mul(a[:], nx[:], c["isx"][:])
        nc.vector.tensor_mul(bh[:], ny[:], c["isyh"][:])
        # bh^2 on ScalarE, accumulated into W2 by the PE array
        nc.scalar.activation(bsq[:], bh[:], AF.Square)
        for k in range(0, FT, 512):
            nc.tensor.matmul(c["w2"][:, k:k + 512], id_sb[:, 0:128],
                             bsq[:, k:k + 512], start=False, stop=True)
        nc.vector.tensor_mul(av[:], a[:], c["t4p1"][:])
        nc.vector.tensor_mul(qn[:], bh[:], t4m1s[:])
        nc.vector.tensor_add(gg[:], av[:], qn[:])          # gv
        nc.vector.tensor_mul(av[:], gg[:], c["st"][:])     # gvs (reuse av)
        nc.vector._custom_dve(
            NLLSUM, out=dead[:], in0=av[:], in1=c["w2"][:],
            s0=0.0, s1=CLAMP, accum_out=part[:, blk:blk + 1],
        )
        del ctxs[blk]

    # Skewed emission (software pipelining): DMA for blk+2, then vector
    # consumers for blk, then front-end producers for blk+1 (so cross-engine
    # chains are emitted producer-first).
    for i in range(NBLK + 2):
        if i < NBLK:
            stage_load(i)
        if 2 <= i and i - 2 < NBLK:
            stage_main(i - 2)
        if 1 <= i and i - 1 < NBLK:
            stage_front(i - 1)

    # loss[p, g] = part[:, 2g] + part[:, 2g+1]
    for g in range(NG):
        nc.vector.tensor_add(
            out_sb[:, g:g + 1], part[:, 2 * g:2 * g + 1],
            part[:, 2 * g + 1:2 * g + 2],
        )
    nc.sync.dma_start(loss, out_sb[:])


_CACHED_NC = None


def _build_program() -> bass.Bass:
    global _CACHED_NC
    if _CACHED_NC is not None:
        return _CACHED_NC
    nc = bacc.Bacc("TRN2", target_bir_lowering=False, debug=False,
                   enable_asserts=False)
    for v in (B_ISYH, B_ST, B_LVC):
        t = nc.alloc_sbuf_tensor(f"const-f32-{v}", [128, 1], F32)
        nc.gpsimd.memset(t.ap(), v)
        nc.const_aps.aps[(F32, v)] = t.ap()
    nc.all_engine_barrier()
    inp = nc.dram_tensor("inp", [NBLK * 128, CW], BF16,
                         kind="ExternalInput").ap()
    ident = nc.dram_tensor("ident", [128, 256], BF16,
                           kind="ExternalInput").ap()
    loss = nc.dram_tensor("loss", [128, NG], F32, kind="ExternalOutput").ap()
    with tile.TileContext(nc) as tc:
        with ExitStack() as ctx:
            _emit(ctx, tc, inp, ident, loss)
    nc.compile()
    _CACHED_NC = nc
    return nc


def _make_ident() -> np.ndarray:
    eye = np.eye(128, dtype=np.float32)
    return np.concatenate([eye, -eye], axis=1).astype(BF16NP)


def make_in_maps(targets: np.ndarray, params: np.ndarray):
    targets = np.asarray(targets, dtype=np.float32)
    params = np.asarray(params, dtype=np.float32)
    ident = _make_ident()
    in_maps = []
    for ci in range(N_CORES):
        sl = slice(ci * PC, (ci + 1) * PC)
        chans = (
            targets[:, sl, 0], targets[:, sl, 1],
            params[:, sl, 0], params[:, sl, 1],
            params[:, sl, 2], params[:, sl, 3], params[:, sl, 4],
        )
        arr = np.empty((NG, NT, 128, 7, FT), dtype=BF16NP)
        for k, ch in enumerate(chans):
            v = ch.astype(BF16NP)                    # [T, PC]
            vv = v.reshape(NT, FT, NG, 128)          # [tc, t', g, p]
            arr[:, :, :, k, :] = vv.transpose(2, 0, 3, 1)
        in_maps.append({"inp": arr.reshape(NBLK * 128, CW), "ident": ident})
    return in_maps


def run_spmd(targets: np.ndarray, params: np.ndarray, trace: bool = False):
    nc = _build_program()
    in_maps = make_in_maps(targets, params)
    res = bass_utils.run_bass_kernel_spmd(
        nc, in_maps, core_ids=list(range(N_CORES)), trace=trace,
    )
    # results[i]["loss"][p, g] is person g*128+p of core i's slice
    loss = np.concatenate(
        [np.asarray(res.results[i]["loss"]).astype(np.float32).T.ravel()
         for i in range(N_CORES)]
    )
    return loss, res


def kernel(targets: np.ndarray, params: np.ndarray,
           peopleIDs: np.ndarray | None = None) -> np.ndarray:
    loss, _ = run_spmd(targets, params, trace=False)
    return loss
